# revision 2
# baseline (speedup 1.0000x reference)
"""CrossAttention kernel v2 for Trainium2 (8 NeuronCores, SPMD).

Reference math (B=4, C=256, N=4096, OUT=256, TEMP=sqrt(OUT)=16):
    q = Wq @ x; k = Wk @ xx; v = Wv @ xx
    attn = softmax(q^T k / TEMP, axis=-1)   (B, N, N)
    y = einsum('bnm,bom->bon', attn, v)     (B, OUT, N)

Sharding: 8 cores = (batch b, query-half h); each core computes its 2048
query rows against the full 4096 keys of its batch.

v2 structure (all matmuls bf16 in / fp32 PSUM accumulate):
    A  = (Wq^T Wk)/TEMP  (host, bf16, C x C)  -- folds q,k projections
    u  = A^T-contracted x: u[c,n] = sum_c' A[c',c] x[c',n]   (C, bc)
    vT = xx^T @ Wv^T (+ones col) -> (m, OUT+1)  [m on partitions]
    S_T = xx-tiles^T @ u -> (m, bc) blocks [m on partitions]  (logits direct)
    P_T = exp(S_T) bf16 (logits are O(2) so no max-subtraction needed)
    yT  = P_T-tiles^T @ vT_aug -> (128n, OUT+1) psum; last col = denom
    y   = yT[:, :OUT] * 1/yT[:, OUT]  -> DMA'd as (bc/128, 128, OUT);
          final (OUT, n) transpose happens on host in gather_output.

The S+exp stage of block b+1 is emitted before the PV stage of block b so
the exp hides under PE matmuls.
"""

import numpy as np
import ml_dtypes
from contextlib import ExitStack

import concourse.bass as bass
import concourse.tile as tile
from concourse import bacc, mybir
from concourse.bass_utils import run_bass_kernel_spmd

B, C, NSEQ, OUT = 4, 256, 4096, 256
TEMP = float(OUT) ** 0.5
NCORES = 8
BF16 = mybir.dt.bfloat16
F32 = mybir.dt.float32
FP8 = mybir.dt.float8e4
BFNP = ml_dtypes.bfloat16
F8NP = ml_dtypes.float8_e4m3
DR = mybir.MatmulPerfMode.DoubleRow

EXP = mybir.ActivationFunctionType.Exp


def build(bc=2048, m=4096, nblk=512, repeat_full=1, dve_pairs=0,
          fp8_blocks=2, fp8_edge_pairs=0):
    """Build the per-core SPMD Bass program.

    bc: query rows per core; m: key count; nblk: query block width
    (nblk*4B <= one PSUM bank). repeat_full: re-run the whole body R times
    (perf measurement only). dve_pairs: of the mt/2 m-tile pairs per block,
    how many run exp on the Vector engine (EXP4_ANT quartic) instead of
    the Activation engine — splits the softmax-exp across both engines.
    fp8_blocks: how many of the nb query blocks compute logits with an
    fp8e4 DoubleRow matmul (2x PE throughput; ~2.1% logit noise, kept
    under the 2e-2 gate by leaving the remaining blocks bf16).
    fp8_edge_pairs: in the last bf16 block, additionally run this many
    trailing m-tile pairs (of mt/2) on the fp8 path — fractional-block
    control of the speed/accuracy trade.
    """
    if dve_pairs:
        from dve_exp4 import EXP4_ANT, D3, D2, D1, D0
    # Logits are computed at 256x scale (A = 16*Wq^T Wk rather than /16)
    # so the fp8e4 operands sit mid-range; exp() folds in the 1/256.
    ESCALE = 1.0 / 256.0
    ct = C // 128     # contraction tiles over channels
    mt = m // 128     # key tiles
    nb = bc // nblk   # query blocks
    nt = nblk // 128  # 128-query tiles per block
    qch = bc // 512
    kch = m // 512

    nbf = nb - fp8_blocks      # leading blocks on the bf16 path
    ubf_cols = nbf * nblk      # query columns needing bf16 u

    nc = bacc.Bacc("TRN2", target_bir_lowering=False, debug=False,
                   num_devices=NCORES)
    x_d = nc.dram_tensor("xq", [ct, 128, bc], BF16, kind="ExternalInput")
    xkv_d = nc.dram_tensor("xkv", [ct, 128, m], BF16, kind="ExternalInput")
    a_d = nc.dram_tensor("aT", [ct, 128, C], BF16, kind="ExternalInput")
    wv_d = nc.dram_tensor("wvT", [ct, 128, OUT], BF16, kind="ExternalInput")
    if fp8_blocks:
        xkv8_d = nc.dram_tensor("xkv8", [ct, 128, m], FP8,
                                kind="ExternalInput")
    y_d = nc.dram_tensor("y", [nb, 128, nt * OUT], BF16,
                         kind="ExternalOutput")

    with tile.TileContext(nc) as tc, ExitStack() as ctx:
        const = ctx.enter_context(tc.tile_pool(name="const", bufs=1))
        # xkv/u/v are read by the S and PV stages at the END of an iteration;
        # double-buffer them so the next iteration's DMA + projections can
        # prefetch during this iteration's attention blocks (keeps the
        # repeat pipeline — and the real single-shot warm path — overlapped).
        dbl = ctx.enter_context(tc.tile_pool(name="dbl", bufs=2))

        x_sb = const.tile([128, ct, bc], BF16, name="x_sb")
        a_sb = const.tile([128, ct, C], BF16, name="a_sb")
        wv_sb = const.tile([128, ct, OUT], BF16, name="wv_sb")
        zbias = const.tile([128, 1], F32, name="zbias")
        nc.vector.memset(zbias[:], 0.0)
        if dve_pairs:
            d0_sb = const.tile([128, 1], F32, name="d0_sb")
            nc.vector.memset(d0_sb[:], D0)

        for _rf in range(repeat_full):
            xkv_sb = dbl.tile([128, ct, m], BF16, tag="xkv", name="xkv_sb")
            v_sb = dbl.tile([128, mt, OUT + 1], BF16, tag="v", name="v_sb")
            if ubf_cols:
                u_sb = dbl.tile([128, ct, ubf_cols], BF16, tag="u",
                                name="u_sb")
            y_sb = dbl.tile([128, nb, nt * OUT], BF16, tag="y", name="y_sb")
            if fp8_blocks:
                xkv8_sb = dbl.tile([128, ct, m], FP8, tag="xkv8",
                                   name="xkv8_sb")
                u8_sb = dbl.tile([128, ct, bc], FP8, tag="u8", name="u8_sb")
            # weights first (small; first matmuls need them), x chunked and
            # c-tiles interleaved so each projection matmul unblocks as soon
            # as its own chunk pair has landed.
            # xkv first: the v projection is the bigger PE chunk and S needs
            # xkv tiles; x (cheap u matmuls) overlaps the tail.
            for i in range(ct):
                nc.sync.dma_start(a_sb[:, i, :], a_d.ap()[i])
                nc.sync.dma_start(wv_sb[:, i, :], wv_d.ap()[i])
            for chk in range(kch):
                for i in range(ct):
                    nc.sync.dma_start(
                        xkv_sb[:, i, chk * 512:(chk + 1) * 512],
                        xkv_d.ap()[i][:, chk * 512:(chk + 1) * 512])
            for chk in range(qch):
                for i in range(ct):
                    nc.sync.dma_start(
                        x_sb[:, i, chk * 512:(chk + 1) * 512],
                        x_d.ap()[i][:, chk * 512:(chk + 1) * 512])
            if fp8_blocks:
                for chk in range(kch):
                    for i in range(ct):
                        nc.sync.dma_start(
                            xkv8_sb[:, i, chk * 512:(chk + 1) * 512],
                            xkv8_d.ap()[i][:, chk * 512:(chk + 1) * 512])
            nc.vector.memset(v_sb[:, :, OUT:OUT + 1], 1.0)

            # ---- v / u projections ----
            with tc.tile_pool(name="u_ps", bufs=3, space="PSUM") as u_pool, \
                 tc.tile_pool(name="v_ps", bufs=3, space="PSUM") as v_pool:
                # v interleaved by xkv chunk so PE work follows DMA arrival
                for chk in range(kch):
                    for mi in range(4 * chk, 4 * (chk + 1)):
                        ps = v_pool.tile([128, OUT], F32, tag="v", name="v_t")
                        for c in range(ct):
                            nc.tensor.matmul(
                                ps[:], xkv_sb[:, c, mi * 128:(mi + 1) * 128],
                                wv_sb[:, c, :],
                                start=(c == 0), stop=(c == ct - 1))
                        nc.scalar.copy(v_sb[:, mi, 0:OUT], ps[:])
                for co in range(ct):
                    for chk in range(qch):
                        ps = u_pool.tile([128, 512], F32, tag="u", name="u_t")
                        for c in range(ct):
                            nc.tensor.matmul(
                                ps[:], a_sb[:, c, co * 128:(co + 1) * 128],
                                x_sb[:, c, chk * 512:(chk + 1) * 512],
                                start=(c == 0), stop=(c == ct - 1))
                        if chk * 512 < ubf_cols:
                            nc.vector.tensor_copy(
                                u_sb[:, co, chk * 512:(chk + 1) * 512],
                                ps[:])
                        if fp8_blocks:
                            nc.vector.tensor_copy(
                                u8_sb[:, co, chk * 512:(chk + 1) * 512],
                                ps[:])

            # ---- attention ----
            with tc.tile_pool(name="p_sb", bufs=2) as p_pool, \
                 tc.tile_pool(name="s_ps", bufs=2, space="PSUM") as s_pool, \
                 tc.tile_pool(name="y_ps", bufs=2, space="PSUM") as y_pool, \
                 tc.tile_pool(name="fin", bufs=8) as fin_pool:
                P_tiles = [None] * nb
                for blk in range(nb + 1):
                    if blk < nb:
                        # S_T = xx^T u for block blk, exp -> P_T
                        # m-tiles paired: one [128, 2, nblk] psum tile
                        # (2 banks), one exp per pair
                        n0 = blk * nblk
                        P_sb = p_pool.tile([128, mt, nblk], BF16, tag="p",
                                           name="P_sb")
                        P_tiles[blk] = P_sb
                        for mj in range(mt // 2):
                            s_ps = s_pool.tile([128, 2, nblk], F32, tag="s",
                                               name="s_t")
                            for half in range(2):
                                mi = 2 * mj + half
                                fp8_here = blk >= nbf or (
                                    fp8_edge_pairs and blk == nbf - 1
                                    and mj >= mt // 2 - fp8_edge_pairs)
                                if fp8_here:
                                    nc.tensor.matmul(
                                        s_ps[:, half, :],
                                        xkv8_sb[:, :, mi * 128:(mi + 1) * 128],
                                        u8_sb[:, :, n0:n0 + nblk],
                                        start=True, stop=True, perf_mode=DR)
                                else:
                                    for c in range(ct):
                                        nc.tensor.matmul(
                                            s_ps[:, half, :],
                                            xkv_sb[:, c,
                                                   mi * 128:(mi + 1) * 128],
                                            u_sb[:, c, n0:n0 + nblk],
                                            start=(c == 0),
                                            stop=(c == ct - 1))
                            if mj >= mt // 2 - dve_pairs:
                                # quartic exp on the Vector engine; 1/256
                                # scale folded into the coefficients
                                nc.vector._custom_dve(
                                    EXP4_ANT,
                                    out=P_sb[:, 2 * mj:2 * mj + 2, :],
                                    in0=s_ps[:], in1=d0_sb[:],
                                    s0=D3 * ESCALE**3, s1=D2 * ESCALE**2,
                                    imm2=D1 * ESCALE)
                            else:
                                nc.scalar.activation(
                                    P_sb[:, 2 * mj:2 * mj + 2, :], s_ps[:],
                                    EXP, bias=zbias[:], scale=ESCALE)
                    if blk == 0:
                        continue
                    # PV stage for block blk-1; normalized output lands in
                    # y_sb (bf16), one batched DMA per block
                    P_sb = P_tiles[blk - 1]
                    for ni in range(nt):
                        y_ps = y_pool.tile([128, OUT + 1], F32, tag="y",
                                           name="y_t")
                        for mi in range(mt):
                            nc.tensor.matmul(
                                y_ps[:],
                                P_sb[:, mi, ni * 128:(ni + 1) * 128],
                                v_sb[:, mi, :],
                                start=(mi == 0), stop=(mi == mt - 1))
                        recip = fin_pool.tile([128, 1], F32, tag="recip",
                                              name="recip")
                        nc.vector.reciprocal(recip[:], y_ps[:, OUT:OUT + 1])
                        nc.vector.tensor_scalar_mul(
                            y_sb[:, blk - 1, ni * OUT:(ni + 1) * OUT],
                            y_ps[:, 0:OUT], recip[:])
                    nc.sync.dma_start(y_d.ap()[blk - 1],
                                      y_sb[:, blk - 1, :])
    nc.compile()
    return nc


def make_in_maps(x, xx, Wq, Wk, Wv, bc=2048, m=4096):
    """Host-side prep: slice/cast per-core inputs. Returns list of 8 dicts."""
    ct = C // 128
    # 256x the logit scale (16*Wq^T@Wk instead of /16): fp8e4 operands sit
    # mid-range; the exp instruction applies 1/256.
    A = (TEMP * (np.asarray(Wq).T @ np.asarray(Wk))).astype(BFNP)  # (C, C)
    a_t = np.ascontiguousarray(A.reshape(ct, 128, C))
    wv_t = np.ascontiguousarray(np.asarray(Wv).T.astype(BFNP)
                                .reshape(ct, 128, OUT))
    halves = NCORES // B
    in_maps = []
    for core in range(NCORES):
        b, h = divmod(core, halves)
        xq = np.ascontiguousarray(
            x[b, :, h * bc:(h + 1) * bc].astype(BFNP).reshape(ct, 128, bc))
        xkv = np.ascontiguousarray(
            xx[b, :, :m].astype(BFNP).reshape(ct, 128, m))
        xkv8 = np.ascontiguousarray(
            xx[b, :, :m].astype(F8NP).reshape(ct, 128, m))
        in_maps.append({"xq": xq, "xkv": xkv, "xkv8": xkv8, "aT": a_t,
                        "wvT": wv_t})
    return in_maps


def gather_output(results, bc=2048, nblk=512):
    """Reassemble per-core y^T outputs into (B, OUT, NSEQ).

    The device emits normalized y^T tiles (queries on partitions) as
    (nb, 128, nt*OUT) bf16; the final (OUT, n) transpose and f32 upcast
    happen here on the host.
    """
    nb, nt = bc // nblk, nblk // 128
    y = np.empty((B, OUT, NSEQ), dtype=np.float32)
    halves = NCORES // B
    for core, res in enumerate(results):
        b, h = divmod(core, halves)
        yt = np.asarray(res["y"], dtype=np.float32)  # (nb, 128, nt*OUT)
        yt = yt.reshape(nb, 128, nt, OUT).transpose(0, 2, 1, 3)
        y[b, :, h * bc:(h + 1) * bc] = yt.reshape(bc, OUT).T
    return y


_NC_CACHE = {}


def kernel(x, xx, Wq, Wk, Wv):
    x = np.asarray(x)
    xx = np.asarray(xx)
    key = "full"
    if key not in _NC_CACHE:
        _NC_CACHE[key] = build()
    nc = _NC_CACHE[key]
    in_maps = make_in_maps(x, xx, np.asarray(Wq), np.asarray(Wk),
                           np.asarray(Wv))
    try:
        res = run_bass_kernel_spmd(nc, in_maps, core_ids=list(range(NCORES)))
    except Exception:
        # transient device state usually clears on retry
        res = run_bass_kernel_spmd(nc, in_maps, core_ids=list(range(NCORES)))
    return gather_output(res.results)
